# revision 1
# baseline (speedup 1.0000x reference)
"""Trainium2 Bass kernel for sliding-window (±64) multi-head attention.

Reference computation (seq=4096, hidden=768, 12 heads x 64, RoPE, window 128):
    qkv = qkv_weight @ x ; q,k = rope(q,k) ; scores = q^T k / 8 + band_mask
    attn = softmax(scores) @ v ; out = out_weight @ attn

Sharding: sequence-parallel over 8 cores. Core c owns queries
[512c, 512c+512) and computes K/V over the haloed span [512c-64, 512c+576)
(zero-padded at the sequence edges; padding is killed by the band mask).
Each core runs an identical Bass program on different data; the full output
is reassembled on host by concatenation (no collectives needed).

Engine notes: DVE/ACT lanes are partition-fixed, so rotate_half (a +-32
partition swap) is done as a PE matmul against a signed permutation matrix;
P^T is produced by a PE matmul against the identity, with the softmax
normalization applied beforehand as a per-partition tensor_scalar multiply.
Attention works on whole head pairs ([128, 512] tiles) to amortize the
per-op access latency of DVE/ACT.
"""

import os
import sys

import numpy as np

for _p in ("/opt/trn_rl_repo",):
    if _p not in sys.path and os.path.isdir(_p):
        sys.path.insert(0, _p)

import ml_dtypes

import concourse.bass as bass
import concourse.bacc as bacc
import concourse.tile as tile
from concourse import mybir
from concourse.bass_utils import run_bass_kernel_spmd

F32 = mybir.dt.float32
F32R = mybir.dt.float32r
BF16 = mybir.dt.bfloat16

N_CORES = 8
SEQ = 4096
S_CORE = SEQ // N_CORES  # 512 queries per core
HALO = 64                # window // 2
SPAN = S_CORE + 2 * HALO  # 640 keys per core
HID = 768
NH = 12
DH = 64
NCH = HID // 128         # 6 contraction chunks
NHP = NH // 2            # 6 head pairs
NQB = S_CORE // 128      # 4 query blocks per core
NSC = SPAN // 128        # 5 key chunks per core
KSPAN = 256              # key span per query block

_BUILD_CACHE = {}


def _build(add_mask: bool, reps: int = 1):
    """Build + compile the per-core Bass program (shared by all 8 cores).

    reps>1 unrolls the whole kernel body (incl. input DMA) that many times
    inside one program — used only by the timing harness.
    """
    nc = bacc.Bacc("TRN2", target_bir_lowering=False, debug=False, num_devices=N_CORES)

    xin = nc.dram_tensor("xin", [128, NCH * SPAN], BF16, kind="ExternalInput")
    wqt = nc.dram_tensor("wqt", [128, NCH * HID], BF16, kind="ExternalInput")
    wkt = nc.dram_tensor("wkt", [128, NCH * HID], BF16, kind="ExternalInput")
    wvt = nc.dram_tensor("wvt", [128, NCH * HID], BF16, kind="ExternalInput")
    wot = nc.dram_tensor("wot", [128, NCH * HID], BF16, kind="ExternalInput")
    cosb = nc.dram_tensor("cosb", [128, SPAN], F32, kind="ExternalInput")
    sinp = nc.dram_tensor("sinp", [128, SPAN], F32, kind="ExternalInput")
    perms = nc.dram_tensor("perms", [128, 128], F32R, kind="ExternalInput")
    maskb = nc.dram_tensor("maskb", [128, NQB * 2 * KSPAN], BF16, kind="ExternalInput")
    if add_mask:
        maskf = nc.dram_tensor(
            "maskf", [128, NQB * 2 * KSPAN], F32, kind="ExternalInput"
        )
    diag = nc.dram_tensor("diag", [128, 128], BF16, kind="ExternalInput")
    out_d = nc.dram_tensor("out", [128, NCH * S_CORE], F32, kind="ExternalOutput")

    mult = mybir.AluOpType.mult
    addop = mybir.AluOpType.add
    exp = mybir.ActivationFunctionType.Exp

    with tile.TileContext(nc) as tc:
        from contextlib import ExitStack

        for _rep in range(reps):
          with ExitStack() as ctx:
            const = ctx.enter_context(tc.tile_pool(name="const", bufs=1))
            sb = ctx.enter_context(tc.tile_pool(name="sb", bufs=1))
            tmp = ctx.enter_context(tc.tile_pool(name="tmp", bufs=4))
            attnp = ctx.enter_context(tc.tile_pool(name="attnp", bufs=6))
            scal = ctx.enter_context(tc.tile_pool(name="scal", bufs=6))
            outp = ctx.enter_context(tc.tile_pool(name="outp", bufs=2))
            ps_proj = ctx.enter_context(
                tc.tile_pool(name="ps_proj", bufs=2, space="PSUM")
            )
            ps_att = ctx.enter_context(
                tc.tile_pool(name="ps_att", bufs=5, space="PSUM")
            )
            ps_o = ctx.enter_context(tc.tile_pool(name="ps_o", bufs=1, space="PSUM"))

            # ---- input DMAs, ordered by first use ----
            # X and WVT per-chunk (VT projection runs first); the rest whole.
            Xc = []
            WVTc = []
            for k in range(NCH):
                xk = const.tile([128, SPAN], BF16, tag=f"X{k}")
                nc.sync.dma_start(out=xk[:], in_=xin.ap()[:, k * SPAN : (k + 1) * SPAN])
                Xc.append(xk)
                wk_ = const.tile([128, HID], BF16, tag=f"WVT{k}")
                nc.sync.dma_start(
                    out=wk_[:], in_=wvt.ap()[:, k * HID : (k + 1) * HID]
                )
                WVTc.append(wk_)
            def load_whp(src_ap, hp, tagpfx):
                t = const.tile([128, NCH * 128], BF16, tag=f"{tagpfx}{hp}")
                nc.sync.dma_start(
                    out=t[:],
                    in_=src_ap[:, hp * NCH * 128 : (hp + 1) * NCH * 128],
                )
                return t

            COS = const.tile([128, SPAN], F32, tag="COS")
            nc.sync.dma_start(out=COS[:], in_=cosb.ap())
            SINP = const.tile([128, SPAN], F32, tag="SINP")
            nc.sync.dma_start(out=SINP[:], in_=sinp.ap())
            PERMS = const.tile([128, 128], F32R, tag="PERMS")
            nc.sync.dma_start(out=PERMS[:], in_=perms.ap())
            MB = const.tile([128, NQB * 2 * KSPAN], BF16, tag="MB")
            nc.sync.dma_start(out=MB[:], in_=maskb.ap())
            if add_mask:
                MF = const.tile([128, NQB * 2 * KSPAN], F32, tag="MF")
                nc.sync.dma_start(out=MF[:], in_=maskf.ap())
            DIAG = const.tile([128, 128], BF16, tag="DIAG")
            nc.sync.dma_start(out=DIAG[:], in_=diag.ap())
            WQc = {}
            WKc = {}
            for hp_ in range(NHP):
                WQc[hp_] = load_whp(wqt.ap(), hp_, "WQ")
                WKc[hp_] = load_whp(wkt.ap(), hp_, "WK")
            WOT = sb.tile([128, NCH * HID], BF16, tag="WOT")
            nc.sync.dma_start(out=WOT[:], in_=wot.ap())

            # persistent intermediates
            Qs = sb.tile([128, NHP * S_CORE], F32R, tag="Qs")   # [2hd, (hp, s)]
            Ks = sb.tile([128, NHP * SPAN], F32R, tag="Ks")     # [2hd, (hp, s)]
            VT = sb.tile([128, NSC * HID], BF16, tag="VT")      # [s, (chunk, hd)]
            AT = sb.tile([128, NCH * S_CORE], BF16, tag="AT")   # [c, (cchunk, s)]

            # ---- V^T projection: VT[s, hd] per 128-key chunk (bf16) ----
            def vt_unit(sc):
                for hf in range(2):
                    w = HID // 2  # 384
                    vp = ps_proj.tile([128, w], F32, tag="proj")
                    for k in range(NCH):
                        nc.tensor.matmul(
                            vp[:],
                            Xc[k][:, sc * 128 : (sc + 1) * 128],
                            WVTc[k][:, hf * w : (hf + 1) * w],
                            start=(k == 0),
                            stop=(k == NCH - 1),
                        )
                    nc.scalar.copy(
                        VT[:, sc * HID + hf * w : sc * HID + (hf + 1) * w], vp[:]
                    )

            def rope(dst, src_ps, cos_ap, sin_ap, w):
                # dst = src*cos + rot(src)*sin ; rot via PE permutation matmul
                qsb = tmp.tile([128, S_CORE], F32R, tag="ropet")
                nc.scalar.copy(qsb[:, :w], src_ps)
                qrot = ps_proj.tile([128, S_CORE], F32, tag="proj")
                nc.tensor.matmul(
                    qrot[:, :w], PERMS[:], qsb[:, :w], start=True, stop=True
                )
                nc.gpsimd.tensor_tensor(dst, qsb[:, :w], cos_ap, op=mult)
                m2 = tmp.tile([128, S_CORE], F32, tag="ropem")
                nc.vector.tensor_tensor(m2[:, :w], qrot[:, :w], sin_ap, op=mult)
                nc.gpsimd.tensor_tensor(dst, dst, m2[:, :w], op=addop)

            # ---- per head pair: project Q,K then attention, software-
            # pipelined: proj(hp+1) is emitted before attention(hp) so the
            # PE queue always has dependency-free matmuls ahead of the
            # attention ops that wait on DVE/ACT results; likewise scores
            # for qb+1 are emitted before the softmax chain of qb. ----
            def proj_hp(hp):
                qp = ps_proj.tile([128, S_CORE], F32, tag="proj")
                for k in range(NCH):
                    nc.tensor.matmul(
                        qp[:],
                        WQc[hp][:, k * 128 : (k + 1) * 128],
                        Xc[k][:, HALO : HALO + S_CORE],
                        start=(k == 0),
                        stop=(k == NCH - 1),
                    )
                rope(
                    Qs[:, hp * S_CORE : (hp + 1) * S_CORE],
                    qp[:],
                    COS[:, HALO : HALO + S_CORE],
                    SINP[:, HALO : HALO + S_CORE],
                    S_CORE,
                )
                for half in range(2):
                    w = SPAN // 2  # 320
                    kp = ps_proj.tile([128, w], F32, tag="proj")
                    for k in range(NCH):
                        nc.tensor.matmul(
                            kp[:],
                            WKc[hp][:, k * 128 : (k + 1) * 128],
                            Xc[k][:, half * w : (half + 1) * w],
                            start=(k == 0),
                            stop=(k == NCH - 1),
                        )
                    rope(
                        Ks[:, hp * SPAN + half * w : hp * SPAN + (half + 1) * w],
                        kp[:],
                        COS[:, half * w : (half + 1) * w],
                        SINP[:, half * w : (half + 1) * w],
                        w,
                    )

            def attn_scores(hp, qb):
                # one PSUM tile (= one bank) per head: the two matmuls
                # contract over different partition row groups and run
                # concurrently on the PE, so they must drain into
                # different PSUM banks.
                ss = []
                for h in range(2):
                    s1 = ps_att.tile([128, KSPAN], F32, tag="att",
                                     name=f"s_{hp}_{qb}_{h}")
                    nc.tensor.matmul(
                        s1[:],
                        Qs[64 * h : 64 * (h + 1),
                           hp * S_CORE + qb * 128 : hp * S_CORE + (qb + 1) * 128],
                        Ks[64 * h : 64 * (h + 1),
                           hp * SPAN + qb * 128 : hp * SPAN + qb * 128 + KSPAN],
                        start=True,
                        stop=True,
                    )
                    ss.append(s1)
                return ss

            # ---- attention as a 6-stage modulo software pipeline over the
            # 24 (head-pair, query-block) units: at each step, stage k runs
            # for unit i-k, so every engine queue holds ready work and ~6
            # units are in flight. ----
            def stage_exp(st):
                praw2 = attnp.tile([128, 2 * KSPAN], BF16, tag="praw")
                moff = st["qb"] * 2 * KSPAN
                for h in range(2):
                    sh = st["s2"][h]
                    dst = praw2[:, h * KSPAN : (h + 1) * KSPAN]
                    if add_mask:
                        ssb2 = tmp.tile([128, KSPAN], F32, tag="ssb")
                        nc.vector.tensor_tensor(
                            ssb2[:], sh[:],
                            MF[:, moff + h * KSPAN : moff + (h + 1) * KSPAN],
                            op=addop,
                        )
                        nc.scalar.activation(dst, ssb2[:], exp)
                    else:
                        nc.scalar.activation(dst, sh[:], exp)
                st["praw"] = praw2
                del st["s2"]

            def stage_dve(st):
                qb = st["qb"]
                moff = qb * 2 * KSPAN
                praw2 = st["praw"]
                P2 = attnp.tile([128, 2 * KSPAN], BF16, tag="P")
                ssum2 = scal.tile([128, 2], F32, tag="ssum")
                nc.vector.tensor_tensor(
                    P2[:], praw2[:], MB[:, moff : moff + 2 * KSPAN], op=mult
                )
                nc.vector.tensor_reduce(
                    out=ssum2[:],
                    in_=P2[:].rearrange("p (h k) -> p h k", h=2),
                    axis=mybir.AxisListType.X,
                    op=addop,
                )
                rr2 = scal.tile([128, 2], F32, tag="rr")
                nc.vector.reciprocal(rr2[:], ssum2[:])
                P2n = attnp.tile([128, 2 * KSPAN], BF16, tag="Pn")
                for h in range(2):
                    nc.vector.tensor_scalar_mul(
                        P2n[:, h * KSPAN : (h + 1) * KSPAN],
                        P2[:, h * KSPAN : (h + 1) * KSPAN],
                        rr2[:, h : h + 1],
                    )
                st["P2n"] = P2n
                del st["praw"]

            def stage_pt(st):
                P2n = st["P2n"]
                pt2 = ps_att.tile([128, 2 * KSPAN], F32, tag="att")
                for h in range(2):
                    for hf in range(2):
                        off = h * KSPAN + hf * 128
                        nc.tensor.matmul(
                            pt2[:, off : off + 128],
                            P2n[:, off : off + 128],
                            DIAG[:],
                            start=True,
                            stop=True,
                        )
                st["pt2"] = pt2
                del st["P2n"]

            def stage_evac(st):
                pts2 = attnp.tile([128, 2 * KSPAN], BF16, tag="pts")
                nc.scalar.copy(pts2[:], st["pt2"][:])
                st["pts2"] = pts2
                del st["pt2"]

            def stage_pv(st):
                hp, qb = st["hp"], st["qb"]
                if qb == 0:
                    o2s[hp] = ps_o.tile([128, S_CORE], F32, tag="o",
                                        name=f"o2_{hp}")
                o2 = o2s[hp]
                pts2 = st["pts2"]
                for h in range(2):
                    hg = hp * 2 + h
                    osl = o2[64 * h : 64 * (h + 1), qb * 128 : (qb + 1) * 128]
                    tp = (0, 64 * h)
                    nc.tensor.matmul(
                        osl,
                        VT[:, qb * HID + hg * 64 : qb * HID + hg * 64 + 64],
                        pts2[:, h * KSPAN : h * KSPAN + 128],
                        start=True, stop=False, tile_position=tp,
                    )
                    nc.tensor.matmul(
                        osl,
                        VT[:, (qb + 1) * HID + hg * 64 : (qb + 1) * HID + hg * 64 + 64],
                        pts2[:, h * KSPAN + 128 : (h + 1) * KSPAN],
                        start=False, stop=True, tile_position=tp,
                    )
                del st["pts2"]
                if qb == NQB - 1:
                    nc.vector.tensor_copy(
                        AT[:, hp * S_CORE : (hp + 1) * S_CORE], o2[:]
                    )
                    del o2s[hp]

            o2s = {}

            def stage_scores(st):
                st["s2"] = attn_scores(st["hp"], st["qb"])

            PO1 = sb.tile([128, NCH * S_CORE], F32, tag="PO1")

            def outproj_part1():
                for oc in range(NCH):
                    ops = ps_proj.tile([128, S_CORE], F32, tag="proj")
                    for k in range(5):
                        nc.tensor.matmul(
                            ops[:],
                            WOT[:, k * HID + oc * 128 : k * HID + (oc + 1) * 128],
                            AT[:, k * S_CORE : (k + 1) * S_CORE],
                            start=(k == 0),
                            stop=(k == 4),
                        )
                    nc.vector.tensor_copy(
                        PO1[:, oc * S_CORE : (oc + 1) * S_CORE], ops[:]
                    )

            vt_unit(0)
            proj_hp(0)
            vt_unit(1)
            proj_hp(1)
            vt_unit(2)
            vt_unit(3)
            vt_unit(4)
            proj_hp(2)

            units = [
                {"hp": hp, "qb": qb} for hp in range(NHP) for qb in range(NQB)
            ]
            stages = [stage_scores, stage_exp, stage_dve, stage_pt,
                      stage_evac, stage_pv]
            NU = len(units)
            ND = len(stages)
            for step in range(NU + ND - 1):
                # emit remaining projections just before each head pair's
                # first unit enters the pipeline
                if step < NU:
                    hp, qb = units[step]["hp"], units[step]["qb"]
                    if qb == 0 and hp + 3 < NHP and hp + 3 >= 3:
                        proj_hp(hp + 3)
                for k in range(ND - 1, -1, -1):
                    idx = step - k
                    if 0 <= idx < NU:
                        stages[k](units[idx])
                # out-projection chunks 0-4 right after head pair 4 retires
                if step == 5 * 4 - 1 + ND - 1:
                    outproj_part1()

            # ---- output projection (split contraction: chunks 0-3 run as
            # soon as head pairs 0-3 are done; 4-5 + combine at the end) ----
            for oc in range(NCH):
                ops = ps_proj.tile([128, S_CORE], F32, tag="proj")
                for k in range(5, NCH):
                    nc.tensor.matmul(
                        ops[:],
                        WOT[:, k * HID + oc * 128 : k * HID + (oc + 1) * 128],
                        AT[:, k * S_CORE : (k + 1) * S_CORE],
                        start=(k == 5),
                        stop=(k == NCH - 1),
                    )
                ot = outp.tile([128, S_CORE], F32, tag="ot")
                nc.vector.scalar_tensor_tensor(
                    out=ot[:], in0=ops[:], scalar=1.0,
                    in1=PO1[:, oc * S_CORE : (oc + 1) * S_CORE],
                    op0=mult, op1=addop,
                )
                nc.sync.dma_start(
                    out=out_d.ap()[:, oc * S_CORE : (oc + 1) * S_CORE], in_=ot[:]
                )

    nc.compile()
    return nc


def get_program(add_mask: bool, reps: int = 1):
    key = (add_mask, reps)
    if key not in _BUILD_CACHE:
        _BUILD_CACHE[key] = _build(add_mask, reps)
    return _BUILD_CACHE[key]


def _pack_chunked(a, nch, w):
    """[nch*128, w] row-major -> [128, nch*w] with chunk-major free dim."""
    return np.ascontiguousarray(
        a.reshape(nch, 128, w).transpose(1, 0, 2).reshape(128, nch * w)
    )


def prep_core_inputs(core, xs, pos, am, qkv_weight, out_weight, add_mask):
    """Build the per-core input map (numpy) for one core."""
    start = S_CORE * core - HALO
    idx = np.arange(start, start + SPAN)
    valid = (idx >= 0) & (idx < SEQ)

    Xs = np.zeros((HID, SPAN), np.float32)
    Xs[:, valid] = xs[:, idx[valid]]

    pspan = np.zeros((SPAN,), np.float32)
    pspan[valid] = pos[idx[valid]]
    invf = (
        1.0 / (10000.0 ** (np.arange(0, DH, 2, dtype=np.float32) / np.float32(DH)))
    ).astype(np.float32)
    f = pspan[None, :] * invf[:, None]  # [32, SPAN]
    cos32 = np.cos(f).astype(np.float32)
    sin32 = np.sin(f).astype(np.float32)
    COS = np.tile(cos32, (4, 1))
    SINP = np.tile(sin32, (4, 1))

    # signed rotate-half permutation: (PERMS.T @ q)[d] = rot_half(q)[d]
    di = np.arange(128)
    lo = (di % 64) < 32
    src = np.where(lo, di + 32, di - 32)
    sgn = np.where(lo, -1.0, 1.0).astype(np.float32)
    PERMS = np.zeros((128, 128), np.float32)
    PERMS[src, di] = sgn

    # masks, duplicated per head of the pair: [128, (qb, h, 256)]
    mb = np.zeros((128, NQB, 2, KSPAN), np.float32)
    mf = np.full((128, NQB, 2, KSPAN), -10000.0, np.float32)
    for qb in range(NQB):
        qg = S_CORE * core + 128 * qb + np.arange(128)
        kg = S_CORE * core + 128 * qb - HALO + np.arange(KSPAN)
        kvalid = (kg >= 0) & (kg < SEQ)
        band = (np.abs(kg[None, :] - qg[:, None]) <= HALO) & kvalid[None, :]
        mb[:, qb, 0, :] = band
        mb[:, qb, 1, :] = band
        if add_mask:
            amband = np.zeros((128, KSPAN), np.float32)
            amband[:, kvalid] = am[np.ix_(qg, kg[kvalid])]
            m = np.where(band, amband, -10000.0)
            mf[:, qb, 0, :] = m
            mf[:, qb, 1, :] = m

    wq = qkv_weight[0:HID] * np.float32(DH**-0.5)
    wk = qkv_weight[HID : 2 * HID]
    wv = qkv_weight[2 * HID : 3 * HID]

    def packw(w):
        return _pack_chunked(
            np.ascontiguousarray(w.T.astype(ml_dtypes.bfloat16)), NCH, HID
        )

    def packw_hp(w):
        # [c, o] -> [128, (hp, cchunk, 128)] so per-head-pair DMAs are
        # contiguous in the free dimension
        wt = np.ascontiguousarray(w.T.astype(ml_dtypes.bfloat16))  # [768c, 768o]
        a = wt.reshape(NCH, 128, NHP, 128)  # (cchunk, p, hp, n)
        return np.ascontiguousarray(
            a.transpose(1, 2, 0, 3).reshape(128, NHP * NCH * 128)
        )

    in_map = {
        "xin": _pack_chunked(Xs.astype(ml_dtypes.bfloat16), NCH, SPAN),
        "wqt": packw_hp(wq),
        "wkt": packw_hp(wk),
        "wvt": packw(wv),
        "wot": packw(out_weight),
        "cosb": COS,
        "sinp": SINP,
        "perms": PERMS,
        "maskb": mb.reshape(128, NQB * 2 * KSPAN).astype(ml_dtypes.bfloat16),
        "diag": np.eye(128, dtype=ml_dtypes.bfloat16),
    }
    if add_mask:
        in_map["maskf"] = np.ascontiguousarray(mf.reshape(128, NQB * 2 * KSPAN))
    return in_map


def prep_all_inputs(x, position_ids, attention_mask, qkv_weight, out_weight):
    xs = np.asarray(x, dtype=np.float32)[0, :, 0, :]  # [768, 4096]
    pos = np.asarray(position_ids)[0].astype(np.float32)
    am = np.asarray(attention_mask, dtype=np.float32)[0, 0]
    qkv_w = np.asarray(qkv_weight, dtype=np.float32)
    out_w = np.asarray(out_weight, dtype=np.float32)
    add_mask = bool(np.any(am))
    in_maps = [
        prep_core_inputs(c, xs, pos, am, qkv_w, out_w, add_mask)
        for c in range(N_CORES)
    ]
    return in_maps, add_mask


def assemble_output(results):
    cols = []
    for c in range(N_CORES):
        o = np.asarray(results[c]["out"])  # [128, 6*512]
        cols.append(o.reshape(128, NCH, S_CORE).transpose(1, 0, 2).reshape(HID, S_CORE))
    full = np.concatenate(cols, axis=1)  # [768, 4096]
    return np.ascontiguousarray(full.reshape(1, HID, 1, SEQ), dtype=np.float32)


def kernel(**inputs):
    in_maps, add_mask = prep_all_inputs(
        inputs["x"],
        inputs["position_ids"],
        inputs["attention_mask"],
        inputs["qkv_weight"],
        inputs["out_weight"],
    )
    nc = get_program(add_mask)
    res = run_bass_kernel_spmd(nc, in_maps, core_ids=list(range(N_CORES)))
    return assemble_output(res.results)



# revision 17
# speedup vs baseline: 1.2457x; 1.2457x over previous
"""Trainium2 Bass kernel for sliding-window (±64) multi-head attention.

Reference (seq=4096, hidden=768, 12 heads x 64, RoPE, window 128):
    qkv = qkv_weight @ x ; q,k = rope(q,k) ; scores = q^T k / 8 + band_mask
    attn = softmax(scores) @ v ; out = out_weight @ attn

Sharding: sequence-parallel over 8 cores. Core c owns queries
[512c, 512c+512) and computes K/V over the haloed span [512c-64, 512c+576)
(zero-padded at the edges; padding killed by the band mask). No collectives.

Design notes (cost model: matmul time = out-free-size only; DVE/ACT/Pool
time = free-size only; DVE 2x for all-bf16-SBUF ops; Pool cannot touch
PSUM; DVE reads at most one PSUM operand):

 - Scores are computed TRANSPOSED, s_T[k, q], by swapping the matmul
   operands (lhsT=K chunk, rhs=Q span). Softmax runs in [k, q] layout:
   exp (ACT), band-mask multiply (DVE, bf16 2x). The P^T transpose of the
   baseline disappears entirely.
 - V^T carries a 65th all-ones column per head, so the PV matmul emits
   the softmax denominators as PSUM row 64 for free. Normalization:
   reciprocal (DVE) -> PE outer-product broadcast (ones_col x recip_row)
   -> ACT evac to SBUF -> DVE multiply.
 - RoPE: head dims are re-laid out (host-side weight row permutation) so
   rotate-half pairs sit 16 partitions apart inside each 32-partition
   block; the rotate is then a single DVE stream_shuffle. Signs are
   folded into the sin table.
 - attention_mask is folded MULTIPLICATIVELY into the band mask
   (exp(s+m) = exp(s)*exp(m)), so one code path covers both.
"""

import os
import sys

import numpy as np

for _p in ("/opt/trn_rl_repo",):
    if _p not in sys.path and os.path.isdir(_p):
        sys.path.insert(0, _p)

import ml_dtypes

import concourse.bass as bass
import concourse.bacc as bacc
import concourse.tile as tile
from concourse import mybir
from concourse.bass_utils import run_bass_kernel_spmd

F32 = mybir.dt.float32
BF16 = mybir.dt.bfloat16
AF = mybir.ActivationFunctionType
ALU = mybir.AluOpType

N_CORES = 8
SEQ = 4096
S_CORE = SEQ // N_CORES  # 512 queries per core
HALO = 64
SPAN = S_CORE + 2 * HALO  # 640 keys per core
HID = 768
NH = 12
DH = 64
NCH = HID // 128          # 6 contraction chunks
NHP = NH // 2             # 6 head pairs
NKC = SPAN // 128         # 5 key chunks
VTW = 65                  # d cols + ones col per head in VT
SHUF16 = [(b + 16) % 32 for b in range(32)]

# scores bank layout: (kc, bank, bank_col, q_start, width)
SEGS = [
    (0, 0, 0, 0, 128),
    (1, 0, 128, 0, 256),
    (4, 0, 384, 384, 128),
    (2, 1, 0, 128, 256),
    (3, 1, 256, 256, 256),
]
# praw column of a (kc) segment = bank*512 + bank_col
KC_PRAW = {kc: b * 512 + bc for kc, b, bc, _, _ in SEGS}
KC_Q0 = {kc: q0 for kc, _, _, q0, _ in SEGS}
# PV sub-matmuls: (q_block, kc, start_flag)
PV_MM = [
    (0, 0, True), (0, 1, False),
    (1, 1, True), (1, 2, False),
    (2, 2, True), (2, 3, False),
    (3, 3, True), (3, 4, False),
]

_BUILD_CACHE = {}


def _build(reps: int = 1):
    nc = bacc.Bacc("TRN2", target_bir_lowering=False, debug=False,
                   num_devices=N_CORES)

    xin = nc.dram_tensor("xin", [128, NCH * SPAN], BF16, kind="ExternalInput")
    wqt = nc.dram_tensor("wqt", [128, NHP * HID], BF16, kind="ExternalInput")
    wkt = nc.dram_tensor("wkt", [128, NHP * HID], BF16, kind="ExternalInput")
    wvt = nc.dram_tensor("wvt", [128, NCH * HID], BF16, kind="ExternalInput")
    wot = nc.dram_tensor("wot", [128, NCH * HID], BF16, kind="ExternalInput")
    cosb = nc.dram_tensor("cosb", [128, SPAN], BF16, kind="ExternalInput")
    sinb = nc.dram_tensor("sinb", [128, SPAN], BF16, kind="ExternalInput")
    maskb = nc.dram_tensor("maskb", [128, 1024], BF16, kind="ExternalInput")
    out_d = nc.dram_tensor("out", [128, NCH * S_CORE], BF16, kind="ExternalOutput")

    with tile.TileContext(nc) as tc:
        from contextlib import ExitStack

        for _rep in range(reps):
          with ExitStack() as ctx:
            const = ctx.enter_context(tc.tile_pool(name="const", bufs=1))
            sb = ctx.enter_context(tc.tile_pool(name="sb", bufs=1))
            rtmp = ctx.enter_context(tc.tile_pool(name="rtmp", bufs=2))
            prawp = ctx.enter_context(tc.tile_pool(name="prawp", bufs=4))
            pp = ctx.enter_context(tc.tile_pool(name="pp", bufs=4))
            rrp = ctx.enter_context(tc.tile_pool(name="rrp", bufs=2))
            rbsp = ctx.enter_context(tc.tile_pool(name="rbsp", bufs=2))
            ntp = ctx.enter_context(tc.tile_pool(name="ntp", bufs=2))
            outp = ctx.enter_context(tc.tile_pool(name="outp", bufs=4))
            ps_proj = ctx.enter_context(
                tc.tile_pool(name="ps_proj", bufs=2, space="PSUM"))
            ps_sc = ctx.enter_context(
                tc.tile_pool(name="ps_sc", bufs=4, space="PSUM"))
            ps_o = ctx.enter_context(
                tc.tile_pool(name="ps_o", bufs=2, space="PSUM"))

            # ---- input DMAs (ordered by first use) ----
            Xc, WVTc = [], []
            for k in range(NCH):
                xk = const.tile([128, SPAN], BF16, tag=f"X{k}")
                nc.sync.dma_start(out=xk[:], in_=xin.ap()[:, k * SPAN:(k + 1) * SPAN])
                Xc.append(xk)
                wk_ = const.tile([128, HID], BF16, tag=f"WVT{k}")
                nc.sync.dma_start(out=wk_[:], in_=wvt.ap()[:, k * HID:(k + 1) * HID])
                WVTc.append(wk_)
            WQc, WKc = {}, {}

            def load_w(src, hp, tagp):
                t = const.tile([128, HID], BF16, tag=f"{tagp}{hp}")
                nc.sync.dma_start(out=t[:], in_=src[:, hp * HID:(hp + 1) * HID])
                return t

            WQc[0] = load_w(wqt.ap(), 0, "WQ")
            WKc[0] = load_w(wkt.ap(), 0, "WK")
            COSB = const.tile([128, SPAN], BF16, tag="COSB")
            nc.sync.dma_start(out=COSB[:], in_=cosb.ap())
            SINB = const.tile([128, SPAN], BF16, tag="SINB")
            nc.sync.dma_start(out=SINB[:], in_=sinb.ap())
            WQc[1] = load_w(wqt.ap(), 1, "WQ")
            WKc[1] = load_w(wkt.ap(), 1, "WK")
            MB = const.tile([128, 1024], BF16, tag="MB")
            nc.sync.dma_start(out=MB[:], in_=maskb.ap())
            for hp_ in range(2, NHP):
                WQc[hp_] = load_w(wqt.ap(), hp_, "WQ")
                WKc[hp_] = load_w(wkt.ap(), hp_, "WK")
            WOT = sb.tile([128, NCH * HID], BF16, tag="WOT")
            nc.sync.dma_start(out=WOT[:], in_=wot.ap())

            # persistent tiles
            Qs = sb.tile([128, NHP * S_CORE], BF16, tag="Qs")
            Ks = sb.tile([128, NHP * SPAN], BF16, tag="Ks")
            VT = sb.tile([128, NKC * NH * VTW], BF16, tag="VT")
            AT = sb.tile([128, NHP * S_CORE], BF16, tag="AT")
            PO1 = sb.tile([128, NCH * S_CORE], F32, tag="PO1")
            ONESL = sb.tile([128, DH], BF16, tag="ONESL")
            nc.vector.memset(ONESL[:], 1.0)
            # ones columns of VT: col 64 of each 65-block
            nc.vector.memset(
                VT[:].rearrange("p (c h w) -> p c h w", c=NKC, h=NH)[:, :, :, 64:65],
                1.0,
            )

            # ---- V^T projection into 65-col head blocks ----
            def vt_unit(kc):
                for hf in range(2):
                    w = HID // 2  # 384 = 6 heads
                    vp = ps_proj.tile([128, w], F32, tag="proj")
                    for k in range(NCH):
                        nc.tensor.matmul(
                            vp[:],
                            Xc[k][:, kc * 128:(kc + 1) * 128],
                            WVTc[k][:, hf * w:(hf + 1) * w],
                            start=(k == 0), stop=(k == NCH - 1),
                        )
                    dst = VT[:].rearrange(
                        "p (c h w) -> p c h w", c=NKC, h=NH
                    )[:, kc, hf * 6:(hf + 1) * 6, 0:64]
                    nc.scalar.copy(dst, vp[:].rearrange(
                        "p (h d) -> p h d", h=6))

            # ---- Q/K projection + rope for one head pair ----
            def proj_hp(hp):
                qp = ps_proj.tile([128, S_CORE], F32, tag="proj")
                for k in range(NCH):
                    nc.tensor.matmul(
                        qp[:],
                        WQc[hp][:, k * 128:(k + 1) * 128],
                        Xc[k][:, HALO:HALO + S_CORE],
                        start=(k == 0), stop=(k == NCH - 1),
                    )
                qsb = rtmp.tile([128, S_CORE], BF16, tag="qsb")
                nc.scalar.copy(qsb[:], qp[:])
                qsh = rtmp.tile([128, S_CORE], BF16, tag="qsh")
                nc.vector.stream_shuffle(qsh[:], qsb[:], mask=SHUF16)
                m1 = rtmp.tile([128, S_CORE], BF16, tag="m1q")
                nc.vector.tensor_tensor(
                    m1[:], qsb[:], COSB[:, HALO:HALO + S_CORE], op=ALU.mult)
                m2 = rtmp.tile([128, S_CORE], BF16, tag="m2q")
                meng = nc.vector if hp >= 4 else nc.gpsimd
                meng.tensor_tensor(
                    m2[:], qsh[:], SINB[:, HALO:HALO + S_CORE], op=ALU.mult)
                nc.vector.tensor_tensor(
                    Qs[:, hp * S_CORE:(hp + 1) * S_CORE], m1[:], m2[:], op=ALU.add)

                ksb = rtmp.tile([128, SPAN], BF16, tag="ksb")
                for half in range(2):
                    w = SPAN // 2  # 320
                    kp = ps_proj.tile([128, w], F32, tag="proj")
                    for k in range(NCH):
                        nc.tensor.matmul(
                            kp[:],
                            WKc[hp][:, k * 128:(k + 1) * 128],
                            Xc[k][:, half * w:(half + 1) * w],
                            start=(k == 0), stop=(k == NCH - 1),
                        )
                    nc.scalar.copy(ksb[:, half * w:(half + 1) * w], kp[:])
                ksh = rtmp.tile([128, SPAN], BF16, tag="ksh")
                nc.vector.stream_shuffle(ksh[:], ksb[:], mask=SHUF16)
                k1 = rtmp.tile([128, SPAN], BF16, tag="m1k")
                nc.vector.tensor_tensor(k1[:], ksb[:], COSB[:], op=ALU.mult)
                k2 = rtmp.tile([128, SPAN], BF16, tag="m2k")
                meng.tensor_tensor(k2[:], ksh[:], SINB[:], op=ALU.mult)
                nc.vector.tensor_tensor(
                    Ks[:, hp * SPAN:(hp + 1) * SPAN], k1[:], k2[:], op=ALU.add)

            # ---- attention stages for unit (hp, h) ----
            def stage_scores(st):
                hp, h = st["hp"], st["h"]
                banks = [ps_sc.tile([128, 512], F32, tag="sc",
                                    name=f"sc_{hp}_{h}_{b}") for b in range(2)]
                for kc, b, bc, q0, w in SEGS:
                    nc.tensor.matmul(
                        banks[b][:, bc:bc + w],
                        Ks[64 * h:64 * (h + 1),
                           hp * SPAN + kc * 128:hp * SPAN + (kc + 1) * 128],
                        Qs[64 * h:64 * (h + 1),
                           hp * S_CORE + q0:hp * S_CORE + q0 + w],
                        start=True, stop=True,
                    )
                st["banks"] = banks

            def stage_exp(st):
                # emitted in the same pipeline step as the scores so the exp
                # sits early in the ACT queue and frees the PSUM banks fast
                praw = prawp.tile([128, 1024], BF16, tag="praw")
                for b in range(2):
                    nc.scalar.activation(
                        praw[:, b * 512:(b + 1) * 512], st["banks"][b][:], AF.Exp)
                st["praw"] = praw
                del st["banks"]

            def stage_mask(st):
                P = pp.tile([128, 1024], BF16, tag="P")
                nc.vector.tensor_tensor(P[:], st["praw"][:], MB[:], op=ALU.mult)
                st["P"] = P
                del st["praw"]

            def stage_pv(st):
                hp, h = st["hp"], st["h"]
                hg = 2 * hp + h
                o65 = ps_o.tile([128, 512], F32, tag="o65",
                                name=f"o65_{hp}_{h}")
                P = st["P"]
                for qb, kc, start in PV_MM:
                    pcol = KC_PRAW[kc] + (qb * 128 - KC_Q0[kc])
                    nc.tensor.matmul(
                        o65[0:65, qb * 128:(qb + 1) * 128],
                        VT[:, kc * NH * VTW + hg * VTW:
                           kc * NH * VTW + hg * VTW + VTW],
                        P[:, pcol:pcol + 128],
                        start=start, stop=(not start),
                    )
                st["o65"] = o65
                del st["P"]

            def stage_epilogue(st):
                # recip of sums row -> PE broadcast into the upper half of
                # the SAME o65 bank -> ACT evac -> DVE normalize.
                hp, h = st["hp"], st["h"]
                o65 = st["o65"]
                rr = rrp.tile([128, 512], BF16, tag="rr")
                with nc.allow_low_precision(reason="softmax reciprocal"):
                    nc.vector.reciprocal(rr[64:65, :], o65[64:65, :])
                nc.tensor.matmul(o65[64:128, :], ONESL[64:65, :],
                                 rr[64:65, :], start=True, stop=True)
                rbs = rbsp.tile([64, 512], BF16, tag="rbs")
                nc.scalar.copy(rbs[0:64, :], o65[64:128, :])
                if h == 0:
                    nc.vector.tensor_tensor(
                        AT[0:64, hp * S_CORE:(hp + 1) * S_CORE],
                        o65[0:64, :], rbs[0:64, :], op=ALU.mult)
                else:
                    nt = ntp.tile([64, 512], BF16, tag="nt")
                    nc.vector.tensor_tensor(
                        nt[0:64, :], o65[0:64, :], rbs[0:64, :], op=ALU.mult)
                    nc.sync.dma_start(
                        out=AT[64:128, hp * S_CORE:(hp + 1) * S_CORE],
                        in_=nt[0:64, :])
                del st["o65"]

            # ---- output projection ----
            def outproj_part1():
                for oc in range(NCH):
                    ops = ps_proj.tile([128, S_CORE], F32, tag="proj")
                    for k in range(5):
                        nc.tensor.matmul(
                            ops[:],
                            WOT[:, k * HID + oc * 128:k * HID + (oc + 1) * 128],
                            AT[:, k * S_CORE:(k + 1) * S_CORE],
                            start=(k == 0), stop=(k == 4),
                        )
                    if oc % 2 == 0:
                        nc.scalar.copy(
                            PO1[:, oc * S_CORE:(oc + 1) * S_CORE], ops[:])
                    else:
                        nc.vector.tensor_copy(
                            PO1[:, oc * S_CORE:(oc + 1) * S_CORE], ops[:])

            def outproj_part2():
                # software-pipelined: mm(oc+1) is emitted before stt(oc) so
                # the PE keeps running while DVE combines.
                banks, ots = {}, {}

                def mm(oc):
                    ops = ps_proj.tile([128, S_CORE], F32, tag="proj")
                    nc.tensor.matmul(
                        ops[:],
                        WOT[:, 5 * HID + oc * 128:5 * HID + (oc + 1) * 128],
                        AT[:, 5 * S_CORE:6 * S_CORE],
                        start=True, stop=True,
                    )
                    banks[oc] = ops

                def fin(oc):
                    ot = outp.tile([128, S_CORE], BF16, tag="ot")
                    nc.vector.scalar_tensor_tensor(
                        out=ot[:], in0=banks.pop(oc)[:], scalar=1.0,
                        in1=PO1[:, oc * S_CORE:(oc + 1) * S_CORE],
                        op0=ALU.mult, op1=ALU.add)
                    nc.sync.dma_start(
                        out=out_d.ap()[:, oc * S_CORE:(oc + 1) * S_CORE],
                        in_=ot[:])

                mm(0)
                for oc in range(1, NCH):
                    mm(oc)
                    fin(oc - 1)
                fin(NCH - 1)

            # ---- schedule: vt/early-proj while input DMAs stream, then the
            # attention pipeline with later projections injected at unit-pair
            # boundaries (they sit BEHIND attention matmuls in the in-order
            # PE queue, keeping the A/D chains fed early). ----
            vt_unit(0)
            vt_unit(1)
            proj_hp(0)
            vt_unit(2)
            proj_hp(1)
            vt_unit(3)

            units = [{"hp": hp, "h": h} for hp in range(NHP) for h in range(2)]

            def stage_scores_exp(st):
                stage_scores(st)
                stage_exp(st)

            def stage_mask_pv(st):
                stage_mask(st)
                stage_pv(st)

            stages = [stage_scores_exp, stage_mask_pv, stage_epilogue]
            NU, ND = len(units), len(stages)
            inject = {0: lambda: vt_unit(4), 1: lambda: proj_hp(2),
                      3: lambda: proj_hp(3), 4: lambda: proj_hp(4),
                      5: lambda: proj_hp(5), 11: outproj_part1}
            for step in range(NU + ND - 1):
                # newest unit's scores go FIRST in the PE queue each step so
                # the PE never sits behind a PV that waits on DVE/ACT results
                for k in range(ND):
                    idx = step - k
                    if 0 <= idx < NU:
                        stages[k](units[idx])
                if step in inject:
                    inject[step]()
            outproj_part2()

    nc.compile()
    return nc


def get_program(add_mask: bool = False, reps: int = 1):
    key = reps
    if key not in _BUILD_CACHE:
        _BUILD_CACHE[key] = _build(reps)
    return _BUILD_CACHE[key]


# ---------------- host-side packing ----------------

# new-layout permutation: position p (0..63) holds old dim PERM64[p]
_p = np.arange(64)
_B = _p // 32
_r = _p % 32
PERM64 = np.where(_r < 16, 16 * _B + _r, 16 * _B + (_r - 16) + 32)
FREQ128 = np.concatenate([16 * _B + (_r % 16)] * 2)  # per-partition freq idx
SGN128 = np.concatenate([np.where(_r < 16, -1.0, 1.0)] * 2).astype(np.float32)


def _pack_chunked(a, nch, w):
    return np.ascontiguousarray(
        a.reshape(nch, 128, w).transpose(1, 0, 2).reshape(128, nch * w))


def prep_core_inputs(core, xs, pos, am, qkv_weight, out_weight, add_mask):
    start = S_CORE * core - HALO
    idx = np.arange(start, start + SPAN)
    valid = (idx >= 0) & (idx < SEQ)

    Xs = np.zeros((HID, SPAN), np.float32)
    Xs[:, valid] = xs[:, idx[valid]]

    pspan = np.zeros((SPAN,), np.float32)
    pspan[valid] = pos[idx[valid]]
    invf = (1.0 / (10000.0 ** (np.arange(0, DH, 2, dtype=np.float32)
                               / np.float32(DH)))).astype(np.float32)
    th = pspan[None, :] * invf[FREQ128][:, None]  # [128, SPAN]
    COSB = np.cos(th).astype(ml_dtypes.bfloat16)
    SINB = (np.sin(th) * SGN128[:, None]).astype(ml_dtypes.bfloat16)

    # band mask in praw layout [k within chunk, (seg cols)]
    mb = np.zeros((128, 1024), np.float32)
    for kc, b, bc, q0, w in SEGS:
        kl = kc * 128 + np.arange(128)
        ql = q0 + np.arange(w)
        kg = start + kl
        kvalid = (kg >= 0) & (kg < SEQ)
        band = ((kl[:, None] - ql[None, :] >= 0)
                & (kl[:, None] - ql[None, :] <= 128)
                & kvalid[:, None])
        m = band.astype(np.float32)
        if add_mask:
            qg = S_CORE * core + ql
            amv = np.zeros((128, w), np.float32)
            amv[kvalid] = am[np.ix_(qg, kg[kvalid])].T[kvalid]
            m = m * np.exp(amv)
        mb[:, b * 512 + bc:b * 512 + bc + w] = m

    wq = qkv_weight[0:HID] * np.float32(DH ** -0.5)
    wk = qkv_weight[HID:2 * HID]
    wv = qkv_weight[2 * HID:3 * HID]

    # permute head-dim rows of wq/wk into the rope-pair layout
    o_perm = (np.arange(NH)[:, None] * DH + PERM64[None, :]).reshape(-1)
    wqp = wq[o_perm]
    wkp = wk[o_perm]

    def packw(w):
        return _pack_chunked(
            np.ascontiguousarray(w.T.astype(ml_dtypes.bfloat16)), NCH, HID)

    def packw_hp(w):
        # [c, o] -> [128, (hp, cchunk, 128)]
        wt = np.ascontiguousarray(w.T.astype(ml_dtypes.bfloat16))
        a = wt.reshape(NCH, 128, NHP, 128)
        return np.ascontiguousarray(
            a.transpose(1, 2, 0, 3).reshape(128, NHP * NCH * 128))

    return {
        "xin": _pack_chunked(Xs.astype(ml_dtypes.bfloat16), NCH, SPAN),
        "wqt": packw_hp(wqp),
        "wkt": packw_hp(wkp),
        "wvt": packw(wv),
        "wot": packw(out_weight),
        "cosb": COSB,
        "sinb": SINB,
        "maskb": mb.astype(ml_dtypes.bfloat16),
    }


def prep_all_inputs(x, position_ids, attention_mask, qkv_weight, out_weight):
    xs = np.asarray(x, dtype=np.float32)[0, :, 0, :]
    pos = np.asarray(position_ids)[0].astype(np.float32)
    am = np.asarray(attention_mask, dtype=np.float32)[0, 0]
    qkv_w = np.asarray(qkv_weight, dtype=np.float32)
    out_w = np.asarray(out_weight, dtype=np.float32)
    add_mask = bool(np.any(am))
    in_maps = [
        prep_core_inputs(c, xs, pos, am, qkv_w, out_w, add_mask)
        for c in range(N_CORES)
    ]
    return in_maps, add_mask


def assemble_output(results):
    cols = []
    for c in range(N_CORES):
        o = np.asarray(results[c]["out"]).astype(np.float32)
        cols.append(o.reshape(128, NCH, S_CORE).transpose(1, 0, 2).reshape(HID, S_CORE))
    full = np.concatenate(cols, axis=1)
    return np.ascontiguousarray(full.reshape(1, HID, 1, SEQ), dtype=np.float32)


def kernel(**inputs):
    in_maps, add_mask = prep_all_inputs(
        inputs["x"], inputs["position_ids"], inputs["attention_mask"],
        inputs["qkv_weight"], inputs["out_weight"],
    )
    nc = get_program(add_mask)
    res = run_bass_kernel_spmd(nc, in_maps, core_ids=list(range(N_CORES)))
    return assemble_output(res.results)
